# revision 6
# baseline (speedup 1.0000x reference)
"""MoE routed expert matmul on 8 Trainium2 NeuronCores.

Problem: out[n] = input[n] @ w[inds[n]] + b[inds[n]]
  input [262144, 32] f32, inds [262144] i32 (1024 experts), w [1024, 32, 32], b [1024, 1, 32]

Strategy (expert-sharded; host does routing/layout only — all FLOPs on device):
  * Host sorts tokens by expert (argsort) and packs, per core k (owning experts
    128k..128k+127), a zero-padded transposed activation tensor with a fixed
    per-expert capacity C (= max expert token count, rounded up, >= 256).
  * Each core runs a fully static Bass/Tile program: per 16-expert group it
    loads one [128, 4C] SBUF tile (features on partitions, 4 row-groups x 4
    column-blocks of experts), then per expert runs one [K=32, M=32, N=C]
    fp32r matmul on PE row-group r (fp32r runs at full rate for N>=256 but
    requires the PSUM destination at partition 0, so each expert gets its own
    PSUM bank).  The PSUM->SBUF copy adds the per-expert bias (split across
    Vector and Scalar engines) and stacks 4 experts back into a [128, C]
    staging tile so outputs leave in big batched DMAs.
  * Host scatters the sorted outputs back to the original token order.

Layouts (core k, local expert e_local = 16g + 4c + r, g<8, c<4, r<4):
  xt [8, 128, 4C]   xt[g, 32r+i, cC+t] = x[token t of expert, feature i]
  wp [128, 1024]    wp[32r+i, (4g+c)*32+o] = w[e, i, o]
  bp [32, 128]      bp[o, e_local]         = b[e, 0, o]
  ot [8, 4, 128, C] ot[g, c, 32r+o, t]     = out[token t of expert, feature o]
"""

import numpy as np

import concourse.bass as bass
import concourse.mybir as mybir
import concourse.tile as tile
from concourse import bacc
from concourse.bass_utils import run_bass_kernel_spmd

N_TOK = 262144
E = 1024
F = 32
O = 32
NCORES = 8
E_LOCAL = E // NCORES  # 128 experts per core
GROUPS = E_LOCAL // 16  # 8 groups of 16 experts
F32 = mybir.dt.float32

MM_DT = mybir.dt.float32r  # matmul operand dtype knob: float32r | bfloat16 | float32

_programs: dict[tuple, "bacc.Bacc"] = {}


def _np_mm_dt():
    return mybir.dt.np(MM_DT)


def _build_program(C: int) -> "bacc.Bacc":
    nc = bacc.Bacc("TRN2", target_bir_lowering=False, debug=False, num_devices=NCORES)
    xt = nc.declare_dram_parameter("xt", [GROUPS, 128, 4 * C], MM_DT, isOutput=False)
    wp = nc.declare_dram_parameter("wp", [128, GROUPS * 4 * O], MM_DT, isOutput=False)
    bp = nc.declare_dram_parameter("bp", [O, E_LOCAL], F32, isOutput=False)
    ot = nc.declare_dram_parameter("ot", [GROUPS, 4, 128, C], F32, isOutput=True)

    with tile.TileContext(nc) as tc:
        with (
            tc.tile_pool(name="w", bufs=1) as w_pool,
            tc.tile_pool(name="xt", bufs=3) as xt_pool,
            tc.tile_pool(name="out", bufs=8) as out_pool,
            tc.tile_pool(name="psum", bufs=8, space="PSUM") as psum_pool,
        ):
            wp_t = w_pool.tile([128, GROUPS * 4 * O], MM_DT)
            nc.sync.dma_start(out=wp_t[:], in_=wp[:])
            bp_t = w_pool.tile([O, E_LOCAL], F32)
            nc.sync.dma_start(out=bp_t[:], in_=bp[:])

            for g in range(GROUPS):
                xt_t = xt_pool.tile([128, 4 * C], MM_DT)
                nc.sync.dma_start(out=xt_t[:], in_=xt[g])

                for c in range(4):
                    o_t = out_pool.tile([128, C], F32, name="o_t", tag="o_t")
                    for r in range(4):
                        e_local = 16 * g + 4 * c + r
                        slot = 4 * g + c
                        psum = psum_pool.tile(
                            [32, C], F32, space="PSUM", name="ps", tag="ps"
                        )
                        nc.tensor.matmul(
                            out=psum[:, :],
                            lhsT=wp_t[32 * r : 32 * r + 32, 32 * slot : 32 * slot + 32],
                            rhs=xt_t[32 * r : 32 * r + 32, c * C : (c + 1) * C],
                            start=True,
                            stop=True,
                            tile_position=(32 * r, 0),
                        )
                        bias_ap = bp_t[:, e_local : e_local + 1]
                        if r % 2 == 0:
                            nc.vector.tensor_scalar_add(
                                o_t[32 * r : 32 * r + 32, :], psum[:, :], bias_ap
                            )
                        else:
                            nc.scalar.activation(
                                o_t[32 * r : 32 * r + 32, :],
                                psum[:, :],
                                mybir.ActivationFunctionType.Identity,
                                bias=bias_ap,
                                scale=1.0,
                            )
                    nc.sync.dma_start(out=ot[g, c], in_=o_t[:])

    nc.compile()
    return nc


def _pack(x, inds, w, b):
    """Host-side routing: sort tokens by expert, build per-core device arrays."""
    counts = np.bincount(inds, minlength=E)
    max_count = int(counts.max())
    C = max(256, -(-max_count // 64) * 64)
    assert C <= 512, f"expert capacity {max_count} exceeds single PSUM bank support"

    order = np.argsort(inds, kind="stable")
    sorted_inds = inds[order]
    starts = np.zeros(E, dtype=np.int64)
    np.cumsum(counts[:-1], out=starts[1:])
    slot = np.arange(N_TOK, dtype=np.int64) - starts[sorted_inds]

    mdt = _np_mm_dt()
    # XT_all[e, i, t] = x[token t of expert e, feature i]
    XT_all = np.zeros((E, F, C), dtype=mdt)
    XT_all[sorted_inds, :, slot] = x[order].astype(mdt)
    # [k, g, c, r, i, t] -> [k, g, (r, i), (c, t)]
    xt = np.ascontiguousarray(
        XT_all.reshape(NCORES, GROUPS, 4, 4, F, C).transpose(0, 1, 3, 4, 2, 5)
    ).reshape(NCORES, GROUPS, 128, 4 * C)

    # [k, g, c, r, i, o] -> [k, (r, i), (g, c, o)]
    wp = np.ascontiguousarray(
        w.astype(mdt).reshape(NCORES, GROUPS, 4, 4, F, O).transpose(0, 3, 4, 1, 2, 5)
    ).reshape(NCORES, 128, GROUPS * 4 * O)

    # bp[k, o, e_local]
    bp = np.ascontiguousarray(b[:, 0, :].reshape(NCORES, E_LOCAL, O).transpose(0, 2, 1))

    return C, order, sorted_inds, slot, xt, wp, bp


def _unpack(results, C, order, sorted_inds, slot):
    # ot[k, g, c, (r, o), t] -> OT_all[e, t, o]
    ot = np.stack([results[k]["ot"] for k in range(NCORES)])
    OT_all = np.ascontiguousarray(
        ot.reshape(NCORES, GROUPS, 4, 4, O, C).transpose(0, 1, 2, 3, 5, 4)
    ).reshape(E, C, O)
    out = np.empty((N_TOK, O), dtype=np.float32)
    out[order] = OT_all[sorted_inds, slot, :]
    return out


def kernel(input, inds, w, b):
    x = np.ascontiguousarray(np.asarray(input, dtype=np.float32))
    inds = np.asarray(inds, dtype=np.int32)
    w = np.ascontiguousarray(np.asarray(w, dtype=np.float32))
    b = np.ascontiguousarray(np.asarray(b, dtype=np.float32))
    assert x.shape == (N_TOK, F) and inds.shape == (N_TOK,)
    assert w.shape == (E, F, O) and b.shape == (E, 1, O)

    C, order, sorted_inds, slot, xt, wp, bp = _pack(x, inds, w, b)

    key = (C, MM_DT)
    nc = _programs.get(key)
    if nc is None:
        nc = _programs[key] = _build_program(C)

    in_maps = [{"xt": xt[k], "wp": wp[k], "bp": bp[k]} for k in range(NCORES)]
    res = run_bass_kernel_spmd(nc, in_maps, list(range(NCORES)))

    return _unpack(res.results, C, order, sorted_inds, slot)


def last_program():
    """The most recently compiled Bass program (for profiling in test.py)."""
    return next(iter(_programs.values())) if _programs else None
